# revision 4
# baseline (speedup 1.0000x reference)
"""Trainium2 Bass kernel for EnergyAwareAdaptiveFusion (moe_routing), v2.

Strategy (v2 — fp8 + DoubleRow rebuild of the fp16 baseline):
  - Only rows with route_choice == 2 need the "full" fusion branch; rows with
    route_choice 0/1 are exact copies of img_emb/txt_emb (assembled on host).
  - Selected rows are gathered, padded to 8*nblocks*NB, data-parallel sharded
    across the 8 NeuronCores (params replicated).  Feature-major layout
    [feature_part, row_free] so every GEMM chains without transposes.
  - The 8 cores share HBM: per-core DMA bytes are the second roofline
    (measured ~128 GB/s/core marginal).  Weights go to 1 byte/param:
      * gate/q/k/v/wo weights: e4m3 (x64 scale), matmul'd in DoubleRow mode
        (2 fp8 weights per PE cell -> 256-deep contraction per instruction,
        ~2x PE throughput).
      * FFN weights: e3m4 (4-bit mantissa, x64) at normal rate, multiplied
        against f16 activations (mixed-dtype matmul) so only the weight
        quantization (~1.8% RMS) touches the FFN — acts stay f16.
  - img/txt DMA'd once as f16 (blend path reads f16); e4m3 copies for the
    DoubleRow moving operands are made on-chip by DVE.
  - Output DMA'd as f16 (host upcasts to f32).
  - seq-len-2 attention: softmax over 2 logits == sigmoid of the scaled score
    difference; ctx-mean collapses Wo to a single GEMM on the mean context.
  - All x64 weight scales are descaled for free inside existing ACT/DVE ops
    (activation scale=, tensor_scalar mult+add).
  - End-to-end quantization error (numpy sim, 3 seeds): max ~1.5e-2 of
    output scale vs the 2e-2 gate.
"""
import numpy as np
import ml_dtypes

import concourse.mybir as mybir
import concourse.tile as tile
from concourse import bacc
from concourse.bass_utils import run_bass_kernel_spmd

P = 128
D = 1024
NF = D // P          # 8 feature tiles
H = 16
NB = 352             # rows per block (must be %16 for DoubleRow AP steps)
NCORES = 8
EPS = 1e-5
WS = 64.0            # weight pre-scale for fp8

f32 = mybir.dt.float32
f16 = mybir.dt.float16
e4 = mybir.dt.float8e4
e3 = mybir.dt.float8e3
E4NP = np.dtype(ml_dtypes.float8_e4m3)
E3NP = np.dtype(ml_dtypes.float8_e3m4)
DR = mybir.MatmulPerfMode.DoubleRow

Act = mybir.ActivationFunctionType
Alu = mybir.AluOpType

GELU_FUNC = Act.Gelu


def _pack_dr(w, scale=WS):
    """[K, M] -> [P, M/128, K/256, 2, P] e4m3 for DoubleRow stationary."""
    K, M = w.shape
    nt, nm = K // 256, M // P
    a = np.clip(np.asarray(w, np.float32) * scale, -224, 224)
    a = a.reshape(nt, 2, P, nm, P).transpose(2, 3, 0, 1, 4)
    return np.ascontiguousarray(a.astype(E4NP))


def _pack_nm(w, scale=WS):
    """[K, M] -> [P, M/128, K/128, P] e3m4 for normal-mode stationary."""
    K, M = w.shape
    nk, nm = K // P, M // P
    a = np.clip(np.asarray(w, np.float32) * scale, -15.0, 15.0)
    a = a.reshape(nk, P, nm, P).transpose(1, 2, 0, 3)
    return np.ascontiguousarray(a.astype(E3NP))


def _pack_b(b, scale=1.0):
    """[M] -> [P, M/128] per-partition f32 bias layout."""
    return np.ascontiguousarray(
        (np.asarray(b, np.float32) * scale).reshape(-1, P).T.astype(np.float32))


def _plan(n2):
    """rows-per-core plan: blocks of <=512 rows, row count %16 (DR AP step)."""
    rows = max(16, -(-n2 // NCORES))
    rows = -(-rows // 16) * 16
    nblocks = max(1, -(-rows // 512))
    nb = -(-rows // nblocks // 16) * 16
    return nblocks, nb


def _build(nblocks, nb=NB, reps=1, stop_after=None, unroll=False, zero_bias=True):
    """Build the per-core program for R = nblocks*nb rows."""
    NB = nb
    R = nblocks * NB
    nc = bacc.Bacc(target_bir_lowering=False, debug=False)

    imgh_d = nc.dram_tensor("imgh", [P, NF, R], f16, kind="ExternalInput")
    txth_d = nc.dram_tensor("txth", [P, NF, R], f16, kind="ExternalInput")
    wg_d = nc.dram_tensor("wg", [P, 8, 8, 2, P], e4, kind="ExternalInput")
    wq_d = nc.dram_tensor("wq", [P, 8, 4, 2, P], e4, kind="ExternalInput")
    wk_d = nc.dram_tensor("wk", [P, 8, 4, 2, P], e4, kind="ExternalInput")
    wv_d = nc.dram_tensor("wv", [P, 8, 4, 2, P], e4, kind="ExternalInput")
    wo_d = nc.dram_tensor("wo", [P, 8, 4, 2, P], e4, kind="ExternalInput")
    wf1_d = nc.dram_tensor("wf1", [P, 32, 8, P], e3, kind="ExternalInput")
    wf2_d = nc.dram_tensor("wf2", [P, 8, 32, P], e3, kind="ExternalInput")
    bg_d = nc.dram_tensor("bg", [P, 8], f32, kind="ExternalInput")
    bq_d = nc.dram_tensor("bq", [P, 8], f32, kind="ExternalInput")     # x64
    bv_d = nc.dram_tensor("bv", [P, 8], f32, kind="ExternalInput")     # x32
    bo_d = nc.dram_tensor("bo", [P, 8], f32, kind="ExternalInput")
    bf1_d = nc.dram_tensor("bf1", [P, 32], f32, kind="ExternalInput")
    bf2_d = nc.dram_tensor("bf2", [P, 8], f32, kind="ExternalInput")
    gamma_d = nc.dram_tensor("gamma", [P, 8], f32, kind="ExternalInput")
    beta_d = nc.dram_tensor("beta", [P, 8], f32, kind="ExternalInput")
    mask_d = nc.dram_tensor("mask", [P, 2], f16, kind="ExternalInput")
    bmaskh_d = nc.dram_tensor("bmaskh", [2, P], f16, kind="ExternalInput")
    ones2_d = nc.dram_tensor("ones2", [P, 2], f16, kind="ExternalInput")
    ones1_d = nc.dram_tensor("ones1", [1, P], f16, kind="ExternalInput")
    out_d = nc.dram_tensor("out", [P, NF, R], f16, kind="ExternalOutput")

    import contextlib
    with tile.TileContext(nc) as tc, contextlib.ExitStack() as ctx:
        ctx.enter_context(nc.allow_low_precision(
            reason="fp8 weights / f16 activations are intentional"))
        if True:
            consts = ctx.enter_context(tc.tile_pool(name="consts", bufs=1))
            pin = ctx.enter_context(tc.tile_pool(name="pin", bufs=1))
            p8 = ctx.enter_context(tc.tile_pool(name="p8", bufs=1))
            pfused = ctx.enter_context(tc.tile_pool(name="pfused", bufs=1))
            pctx = ctx.enter_context(tc.tile_pool(name="pctx", bufs=1))
            # NOTE: ph bufs=2 — a single h buffer reused across row-blocks
            # faulted the device in the fp16 predecessor kernel.
            ph = ctx.enter_context(tc.tile_pool(name="ph", bufs=2))
            pw = ctx.enter_context(tc.tile_pool(name="pw", bufs=6))
            pt = ctx.enter_context(tc.tile_pool(name="pt", bufs=6))
            pq = ctx.enter_context(tc.tile_pool(name="pq", bufs=12))
            psm = ctx.enter_context(tc.tile_pool(name="psm", bufs=8))
            # PSUM: 5 big-matmul banks + 3 stat/broadcast banks = 8 total.
            psum = ctx.enter_context(tc.tile_pool(name="psum", bufs=5, space="PSUM"))
            pst = ctx.enter_context(tc.tile_pool(name="pst", bufs=3, space="PSUM"))

            # constants / params (load once per rep; tiny)
            bg_sb = consts.tile([P, 8], f32)
            nc.sync.dma_start(out=bg_sb, in_=bg_d[:, :])
            bq_sb = consts.tile([P, 8], f32)
            nc.sync.dma_start(out=bq_sb, in_=bq_d[:, :])
            bv_sb = consts.tile([P, 8], f32)
            nc.sync.dma_start(out=bv_sb, in_=bv_d[:, :])
            bo_sb = consts.tile([P, 8], f32)
            nc.sync.dma_start(out=bo_sb, in_=bo_d[:, :])
            bf1_sb = consts.tile([P, 32], f32)
            nc.sync.dma_start(out=bf1_sb, in_=bf1_d[:, :])
            bf2_sb = consts.tile([P, 8], f32)
            nc.sync.dma_start(out=bf2_sb, in_=bf2_d[:, :])
            gamma_sb = consts.tile([P, 8], f32)
            nc.sync.dma_start(out=gamma_sb, in_=gamma_d[:, :])
            beta_sb = consts.tile([P, 8], f32)
            nc.sync.dma_start(out=beta_sb, in_=beta_d[:, :])
            mask_sb = consts.tile([P, 2], f16)
            nc.sync.dma_start(out=mask_sb, in_=mask_d[:, :])
            bmaskh_sb = consts.tile([2, P], f16)
            nc.sync.dma_start(out=bmaskh_sb, in_=bmaskh_d[:, :])
            ones2_sb = consts.tile([P, 2], f16)
            nc.sync.dma_start(out=ones2_sb, in_=ones2_d[:, :])
            ones1_sb = consts.tile([1, P], f16)
            nc.sync.dma_start(out=ones1_sb, in_=ones1_d[:, :])
            eps_sb = consts.tile([1, 1], f32)
            nc.vector.memset(eps_sb, EPS)

            def emit_group(blocks):
                """Weight-major: each weight chunk DMA'd once, used for every
                block in the group."""
                n = len(blocks)
                # tiny dummy op: pulls the sigmoid act-table load off the
                # stage-A critical path (tables swap per function set)
                dsg = psm.tile([1, 1], f32, tag="sc")
                nc.scalar.activation(dsg, eps_sb, Act.Sigmoid)
                imghs, txths, img8s, txt8s, d8s, dbs = [], [], [], [], [], []
                fuseds, ctx8s = [], []
                for j, b in enumerate(blocks):
                    bs, be = b * NB, (b + 1) * NB
                    im = pin.tile([P, NF, NB], f16, tag=f"imgh{j}", name=f"imgh{j}")
                    nc.scalar.dma_start(out=im, in_=imgh_d[:, :, bs:be])
                    tx = pin.tile([P, NF, NB], f16, tag=f"txth{j}", name=f"txth{j}")
                    nc.scalar.dma_start(out=tx, in_=txth_d[:, :, bs:be])
                    imghs.append(im)
                    txths.append(tx)
                    img8s.append(p8.tile([P, NF, NB], e4, tag=f"img8{j}", name=f"img8{j}"))
                    txt8s.append(p8.tile([P, NF, NB], e4, tag=f"txt8{j}", name=f"txt8{j}"))
                    d8s.append(p8.tile([P, NF, NB], e4, tag=f"d8{j}", name=f"d8{j}"))
                    dbs.append(pfused.tile([P, NF, NB], f16, tag=f"db{j}", name=f"db{j}"))
                    fuseds.append(pfused.tile([P, NF, NB], f16, tag=f"fused{j}", name=f"fused{j}"))
                    ctx8s.append(pctx.tile([P, NF, NB], e4, tag=f"ctx8{j}", name=f"ctx8{j}"))

                # on-chip e4m3 copies for the DoubleRow moving operands
                for j in range(n):
                    nc.vector.tensor_copy(img8s[j], imghs[j])
                    nc.vector.tensor_copy(txt8s[j], txths[j])
                    nc.vector.tensor_sub(dbs[j], imghs[j], txths[j])
                    nc.vector.tensor_copy(d8s[j], dbs[j])

                def dump(ap_by_jm, nf=NF):
                    for j, b in enumerate(blocks):
                        bs, be = b * NB, (b + 1) * NB
                        for m in range(nf):
                            o_t = pt.tile([P, NB], f16, tag="t")
                            nc.vector.tensor_copy(o_t, ap_by_jm(j, m))
                            nc.scalar.dma_start(out=out_d[:, m % NF, bs:be],
                                              in_=o_t)

                # ---- stage A: gate = sigmoid(([img|txt] @ Wg)/64 + bg); blend
                for m in range(NF):
                    wgc = pw.tile([P, 8, 2, P], e4, tag="w")
                    nc.sync.dma_start(out=wgc, in_=wg_d[:, m, :, :, :])
                    psA = [psum.tile([P, NB], f32, tag="mm", name=f"psA{jj}")
                           for jj in range(n)]
                    for t in range(8):
                        for j in range(n):
                            src = (img8s[j][:, 2 * t:2 * t + 2, :] if t < 4 else
                                   txt8s[j][:, 2 * (t - 4):2 * (t - 4) + 2, :])
                            nc.tensor.matmul(psA[j], wgc[:, t, :, :], src,
                                             start=(t == 0), stop=(t == 7),
                                             perf_mode=DR)
                    for j in range(n):
                        gate_t = pt.tile([P, NB], f16, tag="t")
                        nc.scalar.activation(gate_t, psA[j], Act.Sigmoid,
                                             bias=bg_sb[:, m:m + 1],
                                             scale=1.0 / WS)
                        gd_t = pt.tile([P, NB], f16, tag="t")
                        nc.vector.tensor_mul(gd_t, gate_t, dbs[j][:, m, :])
                        nc.vector.tensor_add(fuseds[j][:, m, :], gd_t,
                                             txths[j][:, m, :])

                if stop_after == "A":
                    dump(lambda j, m: fuseds[j][:, m, :])
                    return

                # ---- stage B: qkv (DoubleRow) + seq-2 attention -> mean ctx
                b_pending = None
                for fi in range(NF):
                    wqc = pw.tile([P, 4, 2, P], e4, tag="w")
                    nc.sync.dma_start(out=wqc, in_=wq_d[:, fi, :, :, :])
                    wkc = pw.tile([P, 4, 2, P], e4, tag="w")
                    nc.sync.dma_start(out=wkc, in_=wk_d[:, fi, :, :, :])
                    wvc = pw.tile([P, 4, 2, P], e4, tag="w")
                    nc.sync.dma_start(out=wvc, in_=wv_d[:, fi, :, :, :])
                    for j in range(n):
                        i8, t8, dd8 = img8s[j], txt8s[j], d8s[j]
                        # kd first: its drain (kd_s) gates the score chain,
                        # so it overlaps the q matmuls
                        ps_kd = psum.tile([P, NB], f32, tag="mm")
                        for t in range(4):
                            nc.tensor.matmul(ps_kd, wkc[:, t, :, :],
                                             dd8[:, 2 * t:2 * t + 2, :],
                                             start=(t == 0), stop=(t == 3),
                                             perf_mode=DR)
                        ps_q0 = psum.tile([P, NB], f32, tag="mm")
                        ps_q1 = psum.tile([P, NB], f32, tag="mm")
                        for t in range(4):
                            st, sp = (t == 0), (t == 3)
                            sl = slice(2 * t, 2 * t + 2)
                            nc.tensor.matmul(ps_q0, wqc[:, t, :, :],
                                             i8[:, sl, :], start=st, stop=sp,
                                             perf_mode=DR)
                            nc.tensor.matmul(ps_q1, wqc[:, t, :, :],
                                             t8[:, sl, :], start=st, stop=sp,
                                             perf_mode=DR)
                        # vd = (img-txt)@Wv computed directly from d8 (the k
                        # bias cancels; avoids an illegal 2-PSUM DVE read)
                        ps_vd = psum.tile([P, NB], f32, tag="mm")
                        ps_v1 = psum.tile([P, NB], f32, tag="mm")
                        for t in range(4):
                            st, sp = (t == 0), (t == 3)
                            sl = slice(2 * t, 2 * t + 2)
                            nc.tensor.matmul(ps_vd, wvc[:, t, :, :],
                                             dd8[:, sl, :], start=st, stop=sp,
                                             perf_mode=DR)
                            nc.tensor.matmul(ps_v1, wvc[:, t, :, :],
                                             t8[:, sl, :], start=st, stop=sp,
                                             perf_mode=DR)
                        # psums carry x64 weight scale; score sigmoid descales
                        if zero_bias:
                            # drain kd once; multiply straight off the q psums
                            kd_s = pq.tile([P, NB], f16, tag="q")
                            nc.scalar.activation(kd_s, ps_kd, Act.Copy)
                            tmp0 = pq.tile([P, NB], f16, tag="q")
                            nc.vector.tensor_mul(tmp0, kd_s, ps_q0)
                            tmp1 = pq.tile([P, NB], f16, tag="q")
                            nc.vector.tensor_mul(tmp1, kd_s, ps_q1)
                        else:
                            q0b = pq.tile([P, NB], f16, tag="q")
                            nc.scalar.activation(q0b, ps_q0, Act.Identity,
                                                 bias=bq_sb[:, fi:fi + 1])
                            q1b = pq.tile([P, NB], f16, tag="q")
                            nc.scalar.activation(q1b, ps_q1, Act.Identity,
                                                 bias=bq_sb[:, fi:fi + 1])
                            tmp0 = pq.tile([P, NB], f16, tag="q")
                            nc.vector.tensor_mul(tmp0, q0b, ps_kd)
                            tmp1 = pq.tile([P, NB], f16, tag="q")
                            nc.vector.tensor_mul(tmp1, q1b, ps_kd)
                        # v1 drained to SBUF so its bank frees before the
                        # (deferred) score matmuls of this iteration run
                        vds = pq.tile([P, NB], f16, tag="q")
                        nc.scalar.activation(vds, ps_vd, Act.Copy)
                        v1s = pq.tile([P, NB], f16, tag="q")
                        if zero_bias:
                            nc.scalar.activation(v1s, ps_v1, Act.Copy)
                        else:
                            nc.scalar.activation(v1s, ps_v1, Act.Identity,
                                                 bias=bv_sb[:, fi:fi + 1],
                                                 scale=0.5)

                        def score_emit(tmp0=tmp0, tmp1=tmp1, vds=vds,
                                       v1s=v1s, fi=fi, j=j):
                            # score + ctx tail: emitted one iteration late so
                            # these PE matmuls never stall the in-order PE
                            # queue waiting on the ACT/DVE drain chain
                            ps_d0 = pst.tile([2, NB], f32, tag="st")
                            nc.tensor.matmul(ps_d0, mask_sb, tmp0, start=True,
                                             stop=True)
                            ps_d1 = pst.tile([2, NB], f32, tag="st")
                            nc.tensor.matmul(ps_d1, mask_sb, tmp1, start=True,
                                             stop=True)
                            a0 = psm.tile([2, NB], f32, tag="sc")
                            nc.scalar.activation(a0, ps_d0, Act.Sigmoid,
                                                 scale=0.125 / (WS * WS))
                            a1 = psm.tile([2, NB], f32, tag="sc")
                            nc.scalar.activation(a1, ps_d1, Act.Sigmoid,
                                                 scale=0.125 / (WS * WS))
                            asum = psm.tile([2, NB], f16, tag="sc")
                            nc.gpsimd.tensor_add(asum, a0, a1)
                            ps_c = pst.tile([P, NB], f32, tag="st")
                            nc.tensor.matmul(ps_c, bmaskh_sb, asum, start=True,
                                             stop=True)
                            ct = pq.tile([P, NB], f16, tag="q")
                            nc.vector.tensor_tensor(ct, vds, ps_c, Alu.mult)
                            # ctx8 = (64 if zero_bias else 32)*ctx_mean, e4m3
                            nc.gpsimd.tensor_add(ctx8s[j][:, fi, :], ct, v1s)

                        if b_pending is not None:
                            b_pending()
                        b_pending = score_emit

                if b_pending is not None:
                    b_pending()
                    b_pending = None

                if stop_after == "B":
                    dump(lambda j, m: ctx8s[j][:, m, :])
                    return

                # ---- Wo on mean context (DoubleRow); residual into fused
                for m in range(NF):
                    woc = pw.tile([P, 4, 2, P], e4, tag="w")
                    nc.sync.dma_start(out=woc, in_=wo_d[:, m, :, :, :])
                    ps_j = [psum.tile([P, NB], f32, tag="mm", name=f"psW{jj}")
                            for jj in range(n)]
                    for t in range(4):
                        st, sp = (t == 0), (t == 3)
                        sl = slice(2 * t, 2 * t + 2)
                        for j in range(n):
                            nc.tensor.matmul(ps_j[j], woc[:, t, :, :],
                                             ctx8s[j][:, sl, :],
                                             start=st, stop=sp, perf_mode=DR)
                    wo_descale = 1.0 / (WS * WS) if zero_bias else 1.0 / (WS * 32.0)
                    for j in range(n):
                        at = pt.tile([P, NB], f16, tag="t")
                        nc.scalar.activation(at, ps_j[j], Act.Identity,
                                             bias=bo_sb[:, m:m + 1],
                                             scale=wo_descale)
                        nc.vector.tensor_add(fuseds[j][:, m, :], at,
                                             fuseds[j][:, m, :])

                if stop_after == "W":
                    dump(lambda j, m: fuseds[j][:, m, :])
                    return

                # ---- LayerNorm (feature reduction via PE ones-matmul);
                # x-hat overwrites fused in place (f16, feeds FFN1)
                # tiny dummy op: pulls the sqrt act-table load off the LN
                # critical chain (overlaps W-stage instead)
                dsq = psm.tile([1, 1], f32, tag="sc")
                nc.scalar.activation(dsq, eps_sb, Act.Sqrt)
                # stats for BOTH blocks first (PE executes in order — emitting
                # j0's broadcast before j1's stats would stall PE on j0's
                # mean/var/sqrt chain)
                ln_mu, ln_sq = [], []
                for j in range(n):
                    fsd = fuseds[j]
                    ps_mu = pst.tile([2, NB], f32, tag="st")
                    for m in range(NF):
                        nc.tensor.matmul(ps_mu, ones2_sb, fsd[:, m, :],
                                         start=(m == 0), stop=(m == NF - 1))
                    ps_sq = pst.tile([2, NB], f32, tag="st")
                    for m in range(NF):
                        x2 = pt.tile([P, NB], f16, tag="t")
                        nc.vector.tensor_mul(x2, fsd[:, m, :], fsd[:, m, :])
                        nc.tensor.matmul(ps_sq, ones2_sb, x2,
                                         start=(m == 0), stop=(m == NF - 1))
                    ln_mu.append(ps_mu)
                    ln_sq.append(ps_sq)
                for j in range(n):
                    fsd = fuseds[j]
                    ps_mu, ps_sq = ln_mu[j], ln_sq[j]
                    mean = psm.tile([1, NB], f32, tag="sc")
                    nc.scalar.activation(mean, ps_mu[0:1, :], Act.Copy,
                                         scale=1.0 / D)
                    ex2 = psm.tile([1, NB], f32, tag="sc")
                    nc.scalar.activation(ex2, ps_sq[0:1, :], Act.Copy,
                                         scale=1.0 / D)
                    var = psm.tile([1, NB], f32, tag="sc")
                    nc.vector.tensor_mul(var, mean, mean)
                    nc.vector.tensor_tensor(var, ex2, var, Alu.subtract)
                    sd = psm.tile([1, NB], f32, tag="sc")
                    nc.scalar.activation(sd, var, Act.Sqrt, bias=eps_sb[0:1, :])
                    rs = psm.tile([1, NB], f16, tag="sc")
                    nc.vector.reciprocal(rs, sd)
                    ms = psm.tile([1, NB], f16, tag="sc")
                    nc.vector.tensor_mul(ms, mean, rs)
                    ps_rsb = pst.tile([P, NB], f32, tag="st")
                    nc.tensor.matmul(ps_rsb, ones1_sb, rs, start=True, stop=True)
                    ps_msb = pst.tile([P, NB], f32, tag="st")
                    nc.tensor.matmul(ps_msb, ones1_sb, ms, start=True, stop=True)
                    rsb_s = pq.tile([P, NB], f16, tag="q")
                    nc.scalar.activation(rsb_s, ps_rsb, Act.Copy)
                    msb_s = pq.tile([P, NB], f16, tag="q")
                    nc.scalar.activation(msb_s, ps_msb, Act.Copy)
                    for m in range(NF):
                        t = pt.tile([P, NB], f16, tag="t")
                        nc.vector.tensor_tensor(t, fsd[:, m, :], rsb_s,
                                                Alu.mult)
                        if zero_bias:
                            # gamma==1, beta==0: x-hat lands directly
                            nc.vector.tensor_tensor(fsd[:, m, :], t, msb_s,
                                                    Alu.subtract)
                        else:
                            t2 = pt.tile([P, NB], f16, tag="t")
                            nc.vector.tensor_tensor(t2, t, msb_s, Alu.subtract)
                            nc.vector.tensor_scalar(fsd[:, m, :], t2,
                                                    gamma_sb[:, m:m + 1],
                                                    beta_sb[:, m:m + 1],
                                                    Alu.mult, Alu.add)

                if stop_after == "L":
                    dump(lambda j, m: fuseds[j][:, m, :])
                    return

                # ---- FFN1: e3m4 weights x f16 x-hat (mixed dtype), gelu
                hs = [ph.tile([P, 32, NB], f16, tag=f"h{jj}", name=f"h{jj}")
                      for jj in range(n)]
                for m in range(32):
                    w1 = pw.tile([P, 8, P], e3, tag="w")
                    nc.sync.dma_start(out=w1, in_=wf1_d[:, m, :, :])
                    ps_j = [psum.tile([P, NB], f32, tag="mm", name=f"psF{jj}")
                            for jj in range(n)]
                    for k in range(8):
                        for j in range(n):
                            nc.tensor.matmul(ps_j[j], w1[:, k, :],
                                             fuseds[j][:, k, :],
                                             start=(k == 0), stop=(k == 7))
                    for j in range(n):
                        nc.scalar.activation(hs[j][:, m, :], ps_j[j], GELU_FUNC,
                                             bias=bf1_sb[:, m:m + 1],
                                             scale=1.0 / WS)

                if stop_after == "F":
                    dump(lambda j, m: hs[j][:, m % 8, :])
                    return

                # ---- FFN2: e3m4 weights x f16 h (mixed dtype)
                for m in range(NF):
                    ps_out = [psum.tile([P, NB], f32, tag="mm", name=f"psO{jj}")
                              for jj in range(n)]
                    for ck in range(4):
                        w2 = pw.tile([P, 8, P], e3, tag="w")
                        nc.sync.dma_start(
                            out=w2, in_=wf2_d[:, m, ck * 8:(ck + 1) * 8, :])
                        for k in range(8):
                            for j in range(n):
                                nc.tensor.matmul(
                                    ps_out[j], w2[:, k, :],
                                    hs[j][:, ck * 8 + k, :],
                                    start=(ck == 0 and k == 0),
                                    stop=(ck == 3 and k == 7))
                    for j, b in enumerate(blocks):
                        bs, be = b * NB, (b + 1) * NB
                        o_t = pt.tile([P, NB], f16, tag="t")
                        nc.scalar.activation(o_t, ps_out[j], Act.Identity,
                                             bias=bf2_sb[:, m:m + 1],
                                             scale=1.0 / WS)
                        nc.scalar.dma_start(out=out_d[:, m, bs:be], in_=o_t)

            groups = [list(range(g, min(g + 2, nblocks)))
                      for g in range(0, nblocks, 2)]

            def emit_all():
                for g in groups:
                    emit_group(g)

            if reps == 1:
                emit_all()
            elif unroll:
                for _ in range(reps):
                    emit_all()
            else:
                with tc.For_i(0, reps, 1):
                    emit_all()

    nc.compile()
    return nc


_programs = {}


def _get_program(nblocks, nb=NB, reps=1, zero_bias=True):
    key = (nblocks, nb, reps, zero_bias)
    if key not in _programs:
        _programs[key] = _build(nblocks, nb, reps, zero_bias=zero_bias)
    return _programs[key]


def _prep_in_maps(img2, txt2, weights, nblocks, nb=NB, zero_bias=True):
    """img2/txt2: [N2P, D] gathered+padded rows. Returns per-core in_maps."""
    R = nblocks * nb
    n2p = NCORES * R

    def to_fm(x):  # [n2p, D] -> [128, NF, n2p] feature-major f16
        return np.ascontiguousarray(
            np.asarray(x, np.float32).astype(np.float16)
            .reshape(n2p, NF, P).transpose(2, 1, 0))

    img_fm = to_fm(img2)
    txt_fm = to_fm(txt2)

    jj = np.arange(2)[None, :]
    pp = np.arange(P)[:, None]
    mask = ((pp // 64) == jj).astype(np.float16)                  # [128, 2]
    bmh = 0.5 if zero_bias else 0.25
    bmaskh = np.ascontiguousarray(bmh * mask.T)                   # [2, 128]
    ones2 = np.ones((P, 2), np.float16)
    ones1 = np.ones((1, P), np.float16)

    shared = dict(
        wg=weights["wg"], wq=weights["wq"], wk=weights["wk"],
        wv=weights["wv"], wo=weights["wo"],
        wf1=weights["wf1"], wf2=weights["wf2"],
        bg=weights["bg"], bq=weights["bq"], bv=weights["bv"],
        bo=weights["bo"], bf1=weights["bf1"], bf2=weights["bf2"],
        gamma=weights["gamma"], beta=weights["beta"],
        mask=mask, bmaskh=bmaskh, ones2=ones2, ones1=ones1,
    )
    in_maps = []
    for c in range(NCORES):
        m = dict(shared)
        m["imgh"] = np.ascontiguousarray(img_fm[:, :, c * R:(c + 1) * R])
        m["txth"] = np.ascontiguousarray(txt_fm[:, :, c * R:(c + 1) * R])
        in_maps.append(m)
    return in_maps


def _pack_weights(Wg, bg, Wqkv, bqkv, Wo, bo, gamma, beta, Wf1, bf1, Wf2, bf2):
    Wqkv = np.asarray(Wqkv)
    bqkv = np.asarray(bqkv)
    return dict(
        wg=_pack_dr(np.asarray(Wg)),
        wq=_pack_dr(Wqkv[:, :D]), wk=_pack_dr(Wqkv[:, D:2 * D]),
        wv=_pack_dr(Wqkv[:, 2 * D:]),
        wo=_pack_dr(np.asarray(Wo)),
        wf1=_pack_nm(np.asarray(Wf1)), wf2=_pack_nm(np.asarray(Wf2)),
        bg=_pack_b(bg), bq=_pack_b(bqkv[:D], WS),
        bv=_pack_b(bqkv[2 * D:], 32.0), bo=_pack_b(bo),
        bf1=_pack_b(bf1), bf2=_pack_b(bf2),
        gamma=_pack_b(gamma), beta=_pack_b(beta),
    )


def _run_spmd_retry(nc, in_maps, cores, tries=3):
    """The brokered chip occasionally reports a transient device error
    (shared tenancy); retry a couple of times before giving up."""
    import time as _time
    for attempt in range(tries):
        try:
            return run_bass_kernel_spmd(nc, in_maps, cores, trace=False)
        except Exception:
            if attempt == tries - 1:
                raise
            _time.sleep(10.0)


def _run_device(img2, txt2, weights, nblocks, nb=NB, reps=1, zero_bias=True):
    nc = _get_program(nblocks, nb, reps, zero_bias)
    in_maps = _prep_in_maps(img2, txt2, weights, nblocks, nb, zero_bias)
    res = _run_spmd_retry(nc, in_maps, list(range(NCORES)))
    R = nblocks * nb
    n2p = NCORES * R
    # [128, NF, R] f16 per core -> [n2p, D] f32
    full = np.empty((n2p, D), np.float32)
    for c in range(NCORES):
        o = res.results[c]["out"]                  # [128, NF, R] f16
        full[c * R:(c + 1) * R] = (
            o.astype(np.float32).transpose(2, 1, 0).reshape(R, D))
    return full


def kernel(img_emb, txt_emb, route_choice, Wg, bg, Wqkv, bqkv, Wo, bo,
           gamma, beta, Wf1, bf1, Wf2, bf2):
    img_emb = np.asarray(img_emb, dtype=np.float32)
    txt_emb = np.asarray(txt_emb, dtype=np.float32)
    route_choice = np.asarray(route_choice)

    out = np.empty_like(img_emb)
    m0 = route_choice == 0
    m1 = route_choice == 1
    m2 = ~(m0 | m1)
    out[m0] = img_emb[m0]
    out[m1] = txt_emb[m1]

    idx2 = np.flatnonzero(m2)
    n2 = idx2.size
    if n2 == 0:
        return out

    nblocks, nb = _plan(n2)
    n2p = NCORES * nblocks * nb
    img2 = np.zeros((n2p, D), np.float32)
    txt2 = np.zeros((n2p, D), np.float32)
    img2[:n2] = img_emb[idx2]
    txt2[:n2] = txt_emb[idx2]

    zero_bias = bool(
        np.all(np.asarray(bg) == 0) and np.all(np.asarray(bqkv) == 0)
        and np.all(np.asarray(bo) == 0) and np.all(np.asarray(bf1) == 0)
        and np.all(np.asarray(bf2) == 0) and np.all(np.asarray(beta) == 0)
        and np.all(np.asarray(gamma) == 1))
    weights = _pack_weights(Wg, bg, Wqkv, bqkv, Wo, bo, gamma, beta,
                            Wf1, bf1, Wf2, bf2)
    full = _run_device(img2, txt2, weights, nblocks, nb, zero_bias=zero_bias)
    out[idx2] = full[:n2]
    return out
